# revision 4
# baseline (speedup 1.0000x reference)
"""Trainium2 Bass kernel for nn_BatchRelationalWithoutLocationsModule.

Math (per batch b, from the reference):
  o = x[b].reshape(c, h*w).T          # [L, c], c=64, L=256
  A = o @ W0[:c]; B = o @ W0[c:]      # [L, 32]
  h1_{ij} = relu(A_j + B_i + b0); h2 = relu(h1 @ W1 + b1); h3 = relu(h2 @ W2 + b2)
  s = sum_{ij} h3; out = relu(s @ Wp + bp) @ Wo + bo

Distribution: pure data parallel, batch 32 -> 4 per core on 8 NeuronCores.

Device layout per core: partitions = 4 groups x 32 hidden. Group g handles
pair-row i = 64g + t. AT4pb [128, 256] = (A^T + b0) replicated over the 4
groups; BT4 [128, 64] column t = B^T[:, 64g+t] stacked by group. Inner loop
over chunks of 4 t-values:
  act1 (x4): h1[:, kL:(k+1)L] = relu(AT4pb + BT4[:, t])  (DVE tensor_scalar
             dual-op add+max; 256-col granularity forced by per-column bias)
  mm1 (x2, N=512, float32r): p1 = W1bd^T h1   (W1bd = blockdiag(W1 x4))
  act2: h2 = relu(p1 + b14)   ([128, 1024] one op; 1/3 of chunks on DVE,
             2/3 on ACT — measured balance point)
  mm2 (x2): p2 = W2bd^T h2
  act3: h3 = relu(p2 + b24) on ACT at [128, 2048] with accum_out pair-sums
Then per batch reduce the chunk sums, fold the 4 groups with a 0/1 matmul,
and run the tiny head MLP on-chip. float32r runs the PE at full rate
(1 cycle/row for N>=256) with ~tf32 precision; end-to-end rel err ~2.7e-4.
"""
import sys
sys.path.insert(0, "/opt/trn_rl_repo")
import numpy as np

B, C, HW, L, H, NG = 32, 64, 16, 256, 32, 4
N_CORES = 8
B_PER_CORE = B // N_CORES
TPB = L // NG          # 64 t-values per batch
TPC = 4                # t-values per chunk
NCH = TPB // TPC

_CACHE = {}


def _build(dyn_reps=1):
    import contextlib
    import concourse.bacc as bacc
    import concourse.mybir as mybir
    import concourse.tile as tile

    f32 = mybir.dt.float32
    f32r = mybir.dt.float32r
    bf16 = mybir.dt.bfloat16
    AF = mybir.ActivationFunctionType
    ALU = mybir.AluOpType
    STEP = 2           # chunks per p2/act3 op ([128, 2048])

    nc = bacc.Bacc("TRN2", target_bir_lowering=False, debug=False)
    P = lambda name, shape, dt=f32, out=False: nc.declare_dram_parameter(
        name, shape, dt, isOutput=out)

    x_in = P("x", [B_PER_CORE, C, L])
    params = {}
    for name, shape, dt in [
        ("W0a4", [C, 128], f32), ("W0b", [C, H], f32), ("b04", [128, 1], f32),
        ("W1bd", [128, 128], f32r), ("W2bd", [128, 128], f32r),
        ("b14", [128, 1], f32), ("b24", [128, 1], f32), ("FOLD", [128, H], f32),
        ("Wp", [H, H], f32), ("bp", [H, 1], f32),
        ("Wo", [H, H], f32), ("bo", [H, 1], f32),
    ]:
        params[name] = (P(name, shape, dt), shape, dt)
    out = P("out", [H, B_PER_CORE], out=True)

    with tile.TileContext(nc) as tc:
        with (
            tc.tile_pool(name="wpool", bufs=1) as wpool,
            tc.tile_pool(name="xpool", bufs=2) as xpool,
            tc.tile_pool(name="atpool", bufs=B_PER_CORE) as atpool,
            tc.tile_pool(name="btpool", bufs=B_PER_CORE) as btpool,
            tc.tile_pool(name="h1pool", bufs=3) as h1pool,
            tc.tile_pool(name="h2pool", bufs=3) as h2pool,
            tc.tile_pool(name="h3pool", bufs=2) as h3pool,
            tc.tile_pool(name="accpool", bufs=B_PER_CORE) as accpool,
            tc.tile_pool(name="spool", bufs=1) as spool,
            tc.tile_pool(name="ps1", bufs=2, space="PSUM") as ps1_pool,
            tc.tile_pool(name="ps2", bufs=1, space="PSUM") as ps2_pool,
        ):
            reps_ctx = (tc.For_i(0, dyn_reps) if dyn_reps > 1
                        else contextlib.nullcontext())
            with reps_ctx:
                _body(nc, tc, mybir, params, x_in, out,
                      wpool, xpool, atpool, btpool, h1pool, h2pool, h3pool,
                      accpool, spool, ps1_pool, ps2_pool)

    nc.compile()
    return nc


def _body(nc, tc, mybir, params, x_in, out,
          wpool, xpool, atpool, btpool, h1pool, h2pool, h3pool,
          accpool, spool, ps1_pool, ps2_pool):
    f32 = mybir.dt.float32
    f32r = mybir.dt.float32r
    bf16 = mybir.dt.bfloat16
    AF = mybir.ActivationFunctionType
    ALU = mybir.AluOpType
    STEP = 2
    if True:
        if True:
            ld = {}
            for name, (ap, shape, dt) in params.items():
                t = wpool.tile(shape, dt, tag=name)
                nc.sync.dma_start(t[:], ap[:])
                ld[name] = t

            S4 = spool.tile([128, B_PER_CORE], f32, tag="S4")

            # per-batch setup: AT4pb/BT4 for all 4 batches stay resident so
            # the chunk loop below can interleave independent batch streams
            ATs, BTs, ACCs = [], [], []
            for b in range(B_PER_CORE):
                xb = xpool.tile([C, L], f32, tag="xb")
                nc.sync.dma_start(xb[:], x_in[b])

                pA = ps1_pool.tile([128, L], f32, tag="p1")
                nc.tensor.matmul(pA[:], ld["W0a4"][:], xb[:],
                                 start=True, stop=True)
                AT4pb = atpool.tile([128, L], f32r, tag="AT4pb",
                                    name=f"AT4pb{b}")
                nc.scalar.activation(AT4pb[:], pA[:], AF.Identity,
                                     bias=ld["b04"][:])

                pB = ps2_pool.tile([128, TPB], f32, tag="p2")
                for g in range(NG):
                    nc.tensor.matmul(
                        pB[32 * g:32 * (g + 1), :], ld["W0b"][:],
                        xb[:, TPB * g:TPB * (g + 1)],
                        start=True, stop=True, tile_position=(0, 32 * g))
                BT4 = btpool.tile([128, TPB], f32, tag="BT4", name=f"BT4_{b}")
                nc.vector.tensor_copy(BT4[:], pB[:])

                acc_cols = accpool.tile([128, NCH], f32, tag="acc",
                                        name=f"acc{b}")
                ATs.append(AT4pb); BTs.append(BT4); ACCs.append(acc_cols)

            for ch0 in range(0, NCH, STEP):
                for b in range(B_PER_CORE):
                    AT4pb, BT4, acc_cols = ATs[b], BTs[b], ACCs[b]
                    p2w = STEP * TPC * L
                    p2 = ps2_pool.tile([128, p2w], f32, tag="p2")
                    for ci, ch in enumerate(range(ch0, ch0 + STEP)):
                        h1 = h1pool.tile([128, TPC * L], f32r, tag="h1")
                        for k in range(TPC):
                            t = ch * TPC + k
                            dst = h1[:, k * L:(k + 1) * L]
                            bias = BT4[:, t:t + 1]
                            nc.vector.tensor_scalar(
                                dst, AT4pb[:], bias, 0.0, ALU.add, ALU.max)

                        p1 = ps1_pool.tile([128, TPC * L], f32, tag="p1")
                        for m in range(TPC * L // 512):
                            nc.tensor.matmul(
                                p1[:, m * 512:(m + 1) * 512], ld["W1bd"][:],
                                h1[:, m * 512:(m + 1) * 512],
                                start=True, stop=True)

                        h2 = h2pool.tile([128, TPC * L], f32r, tag="h2")
                        if ch % 3 == 0:
                            nc.vector.tensor_scalar(
                                h2[:], p1[:], ld["b14"][:], 0.0,
                                ALU.add, ALU.max)
                        else:
                            nc.scalar.activation(h2[:], p1[:], AF.Relu,
                                                 bias=ld["b14"][:])

                        off = ci * TPC * L
                        for m in range(TPC * L // 512):
                            nc.tensor.matmul(
                                p2[:, off + m * 512:off + (m + 1) * 512],
                                ld["W2bd"][:],
                                h2[:, m * 512:(m + 1) * 512],
                                start=True, stop=True)

                    # h3 is dead data (only accum_out is consumed); bf16 out
                    # halves the ACT SBUF write traffic, accum stays fp32
                    h3 = h3pool.tile([128, p2w], bf16, tag="h3")
                    nc.scalar.activation(
                        h3[:], p2[:], AF.Relu, bias=ld["b24"][:],
                        accum_out=acc_cols[:, ch0 // STEP:ch0 // STEP + 1])

            for b in range(B_PER_CORE):
                nc.vector.tensor_reduce(
                    S4[:, b:b + 1], ACCs[b][:, 0:NCH // STEP],
                    axis=mybir.AxisListType.X, op=ALU.add)

            pS = ps1_pool.tile([H, B_PER_CORE], f32, tag="p1")
            nc.tensor.matmul(pS[:], ld["FOLD"][:], S4[:], start=True, stop=True)
            sT = spool.tile([H, B_PER_CORE], f32, tag="sT")
            nc.vector.tensor_copy(sT[:], pS[:])

            pF = ps2_pool.tile([H, B_PER_CORE], f32, tag="p2")
            nc.tensor.matmul(pF[:], ld["Wp"][:], sT[:], start=True, stop=True)
            fT = spool.tile([H, B_PER_CORE], f32, tag="fT")
            nc.scalar.activation(fT[:], pF[:], AF.Relu, bias=ld["bp"][:])

            pO = ps1_pool.tile([H, B_PER_CORE], f32, tag="p1")
            nc.tensor.matmul(pO[:], ld["Wo"][:], fT[:], start=True, stop=True)
            oT = spool.tile([H, B_PER_CORE], f32, tag="oT")
            nc.scalar.activation(oT[:], pO[:], AF.Identity, bias=ld["bo"][:])

            nc.sync.dma_start(out[:], oT[:])


def _prep_weights(W0, b0, W1, b1, W2, b2, Wp, bp, Wo, bo):
    W0 = np.asarray(W0, np.float32)
    W0a, W0b = W0[:C], W0[C:]
    bd = lambda W: np.kron(np.eye(NG, dtype=np.float32), np.asarray(W, np.float32))
    return {
        "W0a4": np.ascontiguousarray(np.tile(W0a, (1, NG))),
        "W0b": np.ascontiguousarray(W0b),
        "b04": np.tile(np.asarray(b0, np.float32), NG)[:, None],
        "W1bd": bd(W1), "W2bd": bd(W2),
        "b14": np.tile(np.asarray(b1, np.float32), NG)[:, None],
        "b24": np.tile(np.asarray(b2, np.float32), NG)[:, None],
        "FOLD": np.tile(np.eye(H, dtype=np.float32), (NG, 1)),
        "Wp": np.asarray(Wp, np.float32),
        "bp": np.asarray(bp, np.float32)[:, None],
        "Wo": np.asarray(Wo, np.float32),
        "bo": np.asarray(bo, np.float32)[:, None],
    }


def kernel(x_img, W0, b0, W1, b1, W2, b2, Wp, bp, Wo, bo):
    if "nc" not in _CACHE:
        _CACHE["nc"] = _build()
    nc = _CACHE["nc"]

    wd = _prep_weights(W0, b0, W1, b1, W2, b2, Wp, bp, Wo, bo)
    x = np.asarray(x_img, np.float32).reshape(B, C, L)
    in_maps = [
        {"x": np.ascontiguousarray(x[c * B_PER_CORE:(c + 1) * B_PER_CORE]), **wd}
        for c in range(N_CORES)
    ]

    from concourse import bass2jax
    results = bass2jax.run_bass_via_pjrt(nc, in_maps, n_cores=N_CORES)
    full = np.concatenate([r["out"].T for r in results], axis=0)  # [32, 32]
    return full.astype(np.float32)



# revision 17
# speedup vs baseline: 1.2459x; 1.2459x over previous
"""Trainium2 Bass kernel for nn_BatchRelationalWithoutLocationsModule.

Math (per batch b, from the reference):
  o = x[b].reshape(c, h*w).T          # [L, c], c=64, L=256
  A = o @ W0[:c]; B = o @ W0[c:]      # [L, 32]
  h1_{ij} = relu(A_j + B_i + b0); h2 = relu(h1 @ W1 + b1); h3 = relu(h2 @ W2 + b2)
  s = sum_{ij} h3; out = relu(s @ Wp + bp) @ Wo + bo
  (sum is over all L^2 ordered pairs)

Distribution: pure data parallel, batch 32 -> 4 per core on 8 NeuronCores.

Device layout per core: partitions = 4 groups x 32 hidden. Group g handles
pair-row i = 64g + t. AT4pb [128, 256] = (A^T + b0) replicated over the 4
groups (bf16); BT4 [128, 64] column t = B^T[:, 64g+t] stacked by group (bf16).
Per chunk (4 t-values):
  act1 (x4): h1[:, kL:(k+1)L] = relu(AT4pb + BT4[:, t])  on DVE
             (bf16 in/out -> 4x packed mode, 2 elem read+write per port-cycle)
  mm1 (x2, N=512, bf16): p1 = W1bd^T h1   (W1bd = blockdiag(W1 x4))
  act2: h2 = relu(p1 + b14)  [128, 1024] -- ACT or DVE per balance pattern
  mm2 (x2): p2 = W2bd^T h2
  act3: h3 = relu(p2 + b24) + accum_out pair-sums -- ACT or DVE per pattern
The PSUM-sourced act2/act3 ops run at 1 elem/lane/cycle on either engine
(ACT 1.2 GHz, DVE 0.96 GHz); the balance pattern splits them ~2:1.
Then per batch reduce the chunk sums, fold the 4 groups with a 0/1 matmul,
and run the tiny head MLP on-chip.
"""
import sys
sys.path.insert(0, "/opt/trn_rl_repo")
import numpy as np

B, C, HW, L, H, NG = 32, 64, 16, 256, 32, 4
N_CORES = 8
B_PER_CORE = B // N_CORES
TPB = L // NG          # 64 t-values per batch
TPC = 4                # t-values per chunk
NCH = TPB // TPC       # 16 chunks per batch

_CACHE = {}

# evac engine pattern: 'A' = Activation engine, 'D' = DVE. Applied to the
# global sequence of act2/act3 ops round-robin.
DEFAULT_CFG = dict(
    evac_pattern=None,      # legacy: per-op engine cycle (overrides chunk_pattern)
    chunk_pattern="ZXYX",   # per-chunk (act2,act3) engine classes:
                            # X=(ACT,DVE) Y=(DVE,ACT) Z=(ACT,ACT) W=(DVE,DVE)
    ps_bufs=2,              # PSUM buffers for each of p1/p2
    h1_bufs=4, h2_bufs=4, h3_bufs=2,
)


def _build(dyn_reps=1, cfg=None):
    import contextlib
    import concourse.bacc as bacc
    import concourse.mybir as mybir
    import concourse.tile as tile

    cfg = dict(DEFAULT_CFG, **(cfg or {}))
    f32 = mybir.dt.float32
    bf16 = mybir.dt.bfloat16

    nc = bacc.Bacc("TRN2", target_bir_lowering=False, debug=False)
    P = lambda name, shape, dt=f32, out=False: nc.declare_dram_parameter(
        name, shape, dt, isOutput=out)

    x_in = P("x", [B_PER_CORE, C, L])
    params = {}
    for name, shape, dt in [
        ("W0a4", [C, 128], f32), ("W0b", [C, H], f32), ("b04", [128, 1], f32),
        ("W1bd", [128, 128], bf16), ("W2bd", [128, 128], bf16),
        ("b14", [128, 1], f32), ("b24", [128, 1], f32), ("FOLD", [128, H], f32),
        ("Wp", [H, H], f32), ("bp", [H, 1], f32),
        ("Wo", [H, H], f32), ("bo", [H, 1], f32),
    ]:
        params[name] = (P(name, shape, dt), shape, dt)
    out = P("out", [H, B_PER_CORE], out=True)

    with tile.TileContext(nc) as tc:
        with (
            tc.tile_pool(name="wpool", bufs=1) as wpool,
            tc.tile_pool(name="xpool", bufs=B_PER_CORE) as xpool,
            tc.tile_pool(name="atpool", bufs=B_PER_CORE) as atpool,
            tc.tile_pool(name="btpool", bufs=B_PER_CORE) as btpool,
            tc.tile_pool(name="h1pool", bufs=cfg["h1_bufs"]) as h1pool,
            tc.tile_pool(name="h2pool", bufs=cfg["h2_bufs"]) as h2pool,
            tc.tile_pool(name="h3pool", bufs=cfg["h3_bufs"]) as h3pool,
            tc.tile_pool(name="accpool", bufs=B_PER_CORE) as accpool,
            tc.tile_pool(name="spool", bufs=1) as spool,
            tc.tile_pool(name="ps1", bufs=cfg["ps_bufs"], space="PSUM") as ps1_pool,
            tc.tile_pool(name="ps2", bufs=cfg["ps_bufs"], space="PSUM") as ps2_pool,
        ):
            reps_ctx = (tc.For_i(0, dyn_reps) if dyn_reps > 1
                        else contextlib.nullcontext())
            with reps_ctx:
                _body(nc, tc, mybir, cfg, params, x_in, out,
                      wpool, xpool, atpool, btpool, h1pool, h2pool, h3pool,
                      accpool, spool, ps1_pool, ps2_pool)

    nc.compile()
    return nc


def _body(nc, tc, mybir, cfg, params, x_in, out,
          wpool, xpool, atpool, btpool, h1pool, h2pool, h3pool,
          accpool, spool, ps1_pool, ps2_pool):
    f32 = mybir.dt.float32
    bf16 = mybir.dt.bfloat16
    AF = mybir.ActivationFunctionType
    ALU = mybir.AluOpType
    CW = TPC * L           # chunk width = 1024

    # Trigger the ACT table load (Relu/Identity set, ~1.3us) immediately so
    # it overlaps the input DMAs instead of stalling the first real act op.
    zcol = spool.tile([128, 1], f32, tag="zcol")
    nc.vector.memset(zcol[:], 0.0)
    warm = spool.tile([128, 1], f32, tag="warm")
    nc.scalar.activation(warm[:], zcol[:], AF.Relu)

    # Param DMAs in critical-path order: batch-0 setup deps first, then the
    # chunk-loop weights, then the head weights.
    ld = {}
    order = ["W0a4", "W0b", "b04", "W1bd", "b14", "W2bd", "b24",
             "FOLD", "Wp", "bp", "Wo", "bo"]
    xbs = []
    for name in order[:3]:
        ap, shape, dt = params[name]
        t = wpool.tile(shape, dt, tag=name)
        nc.sync.dma_start(t[:], ap[:])
        ld[name] = t
    for b in range(B_PER_CORE):
        xb = xpool.tile([C, L], f32, tag="xb", name=f"xb{b}")
        nc.sync.dma_start(xb[:], x_in[b])
        xbs.append(xb)
    for name in order[3:]:
        ap, shape, dt = params[name]
        t = wpool.tile(shape, dt, tag=name)
        nc.sync.dma_start(t[:], ap[:])
        ld[name] = t

    S4 = spool.tile([128, B_PER_CORE], f32, tag="S4")

    # Per-batch setup: AT4pb (A^T + b0, replicated x4, bf16) and BT4
    # (B^T stacked by group, bf16) stay resident for the chunk loop.
    ATs, BTs, ACCs = [], [], []
    for b in range(B_PER_CORE):
        xb = xbs[b]

        pA = ps1_pool.tile([128, L], f32, tag="p1")
        nc.tensor.matmul(pA[:], ld["W0a4"][:], xb[:], start=True, stop=True)
        AT4pb = atpool.tile([128, L], bf16, tag="AT4pb", name=f"AT4pb{b}")
        nc.scalar.activation(AT4pb[:], pA[:], AF.Identity, bias=ld["b04"][:])

        pB = ps2_pool.tile([128, TPB], f32, tag="p2")
        for g in range(NG):
            nc.tensor.matmul(
                pB[32 * g:32 * (g + 1), :], ld["W0b"][:],
                xb[:, TPB * g:TPB * (g + 1)],
                start=True, stop=True, tile_position=(0, 32 * g))
        BT4 = btpool.tile([128, TPB], f32, tag="BT4", name=f"BT4_{b}")
        nc.vector.tensor_copy(BT4[:], pB[:])

        acc_cols = accpool.tile([128, NCH], f32, tag="acc", name=f"acc{b}")
        ATs.append(AT4pb); BTs.append(BT4); ACCs.append(acc_cols)

    CLS = {"X": "AD", "Y": "DA", "Z": "AA", "W": "DD"}
    if cfg.get("evac_pattern") or cfg.get("act2_pattern"):
        pat2 = cfg.get("act2_pattern") or cfg["evac_pattern"]
        pat3 = cfg.get("act3_pattern") or cfg["evac_pattern"]
    else:
        cp = cfg["chunk_pattern"]
        pat2 = "".join(CLS[c][0] for c in cp)
        pat3 = "".join(CLS[c][1] for c in cp)
    g2 = g3 = 0  # per-stage chunk counters for the engine patterns

    def evac(dst, src, bias, accum=None):
        nonlocal g2, g3
        if accum is not None:
            eng = pat3[g3 % len(pat3)]
            g3 += 1
        else:
            eng = pat2[g2 % len(pat2)]
            g2 += 1
        if eng == "D":
            if accum is None:
                nc.vector.tensor_scalar(dst, src, bias, 0.0, ALU.add, ALU.max)
            else:
                # relu+bias+row-sum in one DVE op: out = (src+bias) max 0,
                # accum_out = sum(out). (tensor_scalar with accum_out would
                # repurpose op1 as the reduce op and drop the relu.)
                nc.vector.scalar_tensor_tensor(
                    dst, src, bias, zcol[:, 0:1].broadcast_to(dst.shape),
                    ALU.add, ALU.max, accum_out=accum)
        else:
            kw = {} if accum is None else {"accum_out": accum}
            nc.scalar.activation(dst, src, AF.Relu, bias=bias, **kw)

    for ch in range(NCH):
        for b in range(B_PER_CORE):
            AT4pb, BT4, acc_cols = ATs[b], BTs[b], ACCs[b]

            h1 = h1pool.tile([128, CW], bf16, tag="h1")
            for k in range(TPC):
                t = ch * TPC + k
                nc.vector.tensor_scalar(
                    h1[:, k * L:(k + 1) * L], AT4pb[:], BT4[:, t:t + 1],
                    0.0, ALU.add, ALU.max)

            p1 = ps1_pool.tile([128, CW], f32, tag="p1")
            for m in range(CW // 512):
                nc.tensor.matmul(
                    p1[:, m * 512:(m + 1) * 512], ld["W1bd"][:],
                    h1[:, m * 512:(m + 1) * 512], start=True, stop=True)

            h2 = h2pool.tile([128, CW], bf16, tag="h2")
            evac(h2[:], p1[:], ld["b14"][:])

            p2 = ps2_pool.tile([128, CW], f32, tag="p2")
            for m in range(CW // 512):
                nc.tensor.matmul(
                    p2[:, m * 512:(m + 1) * 512], ld["W2bd"][:],
                    h2[:, m * 512:(m + 1) * 512], start=True, stop=True)

            h3 = h3pool.tile([128, CW], bf16, tag="h3")
            evac(h3[:], p2[:], ld["b24"][:], accum=acc_cols[:, ch:ch + 1])

    for b in range(B_PER_CORE):
        nc.vector.tensor_reduce(
            S4[:, b:b + 1], ACCs[b][:, 0:NCH],
            axis=mybir.AxisListType.X, op=ALU.add)

    pS = ps1_pool.tile([H, B_PER_CORE], f32, tag="p1")
    nc.tensor.matmul(pS[:], ld["FOLD"][:], S4[:], start=True, stop=True)
    sT = spool.tile([H, B_PER_CORE], f32, tag="sT")
    nc.vector.tensor_copy(sT[:], pS[:])

    pF = ps2_pool.tile([H, B_PER_CORE], f32, tag="p2")
    nc.tensor.matmul(pF[:], ld["Wp"][:], sT[:], start=True, stop=True)
    fT = spool.tile([H, B_PER_CORE], f32, tag="fT")
    nc.scalar.activation(fT[:], pF[:], AF.Relu, bias=ld["bp"][:])

    pO = ps1_pool.tile([H, B_PER_CORE], f32, tag="p1")
    nc.tensor.matmul(pO[:], ld["Wo"][:], fT[:], start=True, stop=True)
    oT = spool.tile([H, B_PER_CORE], f32, tag="oT")
    nc.scalar.activation(oT[:], pO[:], AF.Identity, bias=ld["bo"][:])

    nc.sync.dma_start(out[:], oT[:])


def _prep_weights(W0, b0, W1, b1, W2, b2, Wp, bp, Wo, bo):
    import ml_dtypes
    bfnp = ml_dtypes.bfloat16
    W0 = np.asarray(W0, np.float32)
    W0a, W0b = W0[:C], W0[C:]
    bd = lambda W: np.kron(np.eye(NG, dtype=np.float32),
                           np.asarray(W, np.float32))
    return {
        "W0a4": np.ascontiguousarray(np.tile(W0a, (1, NG))),
        "W0b": np.ascontiguousarray(W0b),
        "b04": np.tile(np.asarray(b0, np.float32), NG)[:, None],
        "W1bd": bd(W1).astype(bfnp), "W2bd": bd(W2).astype(bfnp),
        "b14": np.tile(np.asarray(b1, np.float32), NG)[:, None],
        "b24": np.tile(np.asarray(b2, np.float32), NG)[:, None],
        "FOLD": np.tile(np.eye(H, dtype=np.float32), (NG, 1)),
        "Wp": np.asarray(Wp, np.float32),
        "bp": np.asarray(bp, np.float32)[:, None],
        "Wo": np.asarray(Wo, np.float32),
        "bo": np.asarray(bo, np.float32)[:, None],
    }


def kernel(x_img, W0, b0, W1, b1, W2, b2, Wp, bp, Wo, bo):
    if "nc" not in _CACHE:
        _CACHE["nc"] = _build()
    nc = _CACHE["nc"]

    wd = _prep_weights(W0, b0, W1, b1, W2, b2, Wp, bp, Wo, bo)
    x = np.asarray(x_img, np.float32).reshape(B, C, L)
    in_maps = [
        {"x": np.ascontiguousarray(x[c * B_PER_CORE:(c + 1) * B_PER_CORE]),
         **wd}
        for c in range(N_CORES)
    ]

    from concourse import bass2jax
    results = bass2jax.run_bass_via_pjrt(nc, in_maps, n_cores=N_CORES)
    full = np.concatenate([r["out"].T for r in results], axis=0)  # [32, 32]
    return full.astype(np.float32)


# revision 31
# speedup vs baseline: 1.3437x; 1.0785x over previous
"""Trainium2 Bass kernel for nn_BatchRelationalWithoutLocationsModule.

Math (per batch b, from the reference):
  o = x[b].reshape(c, h*w).T          # [L, c], c=64, L=256
  A = o @ W0[:c]; B = o @ W0[c:]      # [L, 32]
  h1_{ij} = relu(A_j + B_i + b0); h2 = relu(h1 @ W1 + b1); h3 = relu(h2 @ W2 + b2)
  s = sum_{ij} h3; out = relu(s @ Wp + bp) @ Wo + bo
  (sum is over all L^2 ordered pairs)

Distribution: pure data parallel, batch 32 -> 4 per core on 8 NeuronCores.

Device layout per core: partitions = 4 groups x 32 hidden. Group g handles
pair-row i = 64g + t. AT4pb [128, 256] = (A^T + b0) replicated over the 4
groups (bf16); BT4 [128, 64] column t = B^T[:, 64g+t] stacked by group (bf16).
Per chunk (4 t-values):
  act1 (x4): h1[:, kL:(k+1)L] = relu(AT4pb + BT4[:, t])  on DVE
             (bf16 in/out -> 4x packed mode, 2 elem read+write per port-cycle)
  mm1 (x2, N=512, bf16): p1 = W1bd^T h1   (W1bd = blockdiag(W1 x4))
  act2: h2 = relu(p1 + b14)  [128, 1024] -- ACT or DVE per balance pattern
  mm2 (x2): p2 = W2bd^T h2
  act3: h3 = relu(p2 + b24) + accum_out pair-sums -- ACT or DVE per pattern
The PSUM-sourced act2/act3 ops run at 1 elem/lane/cycle on either engine
(ACT 1.2 GHz, DVE 0.96 GHz); the balance pattern splits them ~2:1.
Then per batch reduce the chunk sums, fold the 4 groups with a 0/1 matmul,
and run the tiny head MLP on-chip.
"""
import sys
sys.path.insert(0, "/opt/trn_rl_repo")
import numpy as np

B, C, HW, L, H, NG = 32, 64, 16, 256, 32, 4
N_CORES = 8
B_PER_CORE = B // N_CORES
TPB = L // NG          # 64 t-values per batch
TPC = 4                # t-values per chunk
NCH = TPB // TPC       # 16 chunks per batch

_CACHE = {}

# evac engine pattern: 'A' = Activation engine, 'D' = DVE. Applied to the
# global sequence of act2/act3 ops round-robin.
DEFAULT_CFG = dict(
    evac_pattern=None,      # legacy: per-op engine cycle (overrides chunk_pattern)
    chunk_pattern="ZXYX",   # per-chunk (act2,act3) engine classes:
                            # X=(ACT,DVE) Y=(DVE,ACT) Z=(ACT,ACT) W=(DVE,DVE)
    ps_bufs=2,              # PSUM buffers for each of p1/p2
    h1_bufs=4, h2_bufs=4, h3_bufs=2,
)


def _build(dyn_reps=1, cfg=None):
    import contextlib
    import concourse.bacc as bacc
    import concourse.mybir as mybir
    import concourse.tile as tile

    cfg = dict(DEFAULT_CFG, **(cfg or {}))
    f32 = mybir.dt.float32
    f32r = mybir.dt.float32r
    bf16 = mybir.dt.bfloat16

    nc = bacc.Bacc("TRN2", target_bir_lowering=False, debug=False)
    P = lambda name, shape, dt=f32, out=False: nc.declare_dram_parameter(
        name, shape, dt, isOutput=out)

    x_in = P("x", [C, B_PER_CORE * L], bf16)
    params = {
        "w0p": (P("w0p", [C, 160], bf16), [C, 160], bf16),
        "wbf": (P("wbf", [128, 256], bf16), [128, 256], bf16),
        "wf": (P("wf", [128, 101], f32), [128, 101], f32),
    }
    out = P("out", [H, B_PER_CORE], out=True)

    with tile.TileContext(nc) as tc:
        with (
            tc.tile_pool(name="wpool", bufs=1) as wpool,
            tc.tile_pool(name="xpool", bufs=B_PER_CORE) as xpool,
            tc.tile_pool(name="atpool", bufs=B_PER_CORE) as atpool,
            tc.tile_pool(name="btpool", bufs=B_PER_CORE) as btpool,
            tc.tile_pool(name="h1pool", bufs=cfg["h1_bufs"]) as h1pool,
            tc.tile_pool(name="h2pool", bufs=cfg["h2_bufs"]) as h2pool,
            tc.tile_pool(name="h3pool", bufs=cfg["h3_bufs"]) as h3pool,
            tc.tile_pool(name="accpool", bufs=B_PER_CORE) as accpool,
            tc.tile_pool(name="spool", bufs=1) as spool,
            tc.tile_pool(name="ps1", bufs=cfg["ps_bufs"], space="PSUM") as ps1_pool,
            tc.tile_pool(name="ps2", bufs=cfg["ps_bufs"], space="PSUM") as ps2_pool,
        ):
            reps_ctx = (tc.For_i(0, dyn_reps) if dyn_reps > 1
                        else contextlib.nullcontext())
            with reps_ctx:
                _body(nc, tc, mybir, cfg, params, x_in, out,
                      wpool, xpool, atpool, btpool, h1pool, h2pool, h3pool,
                      accpool, spool, ps1_pool, ps2_pool)

    nc.compile()
    return nc


def _body(nc, tc, mybir, cfg, params, x_in, out,
          wpool, xpool, atpool, btpool, h1pool, h2pool, h3pool,
          accpool, spool, ps1_pool, ps2_pool):
    f32 = mybir.dt.float32
    f32r = mybir.dt.float32r
    bf16 = mybir.dt.bfloat16
    AF = mybir.ActivationFunctionType
    ALU = mybir.AluOpType
    CW = TPC * L           # chunk width = 1024

    # Trigger the ACT table load (Relu/Identity set, ~1.3us) immediately so
    # it overlaps the input DMAs instead of stalling the first real act op.
    zcol = spool.tile([128, 1], f32, tag="zcol")
    nc.vector.memset(zcol[:], 0.0)
    nc.scalar.activation(zcol[:], zcol[:], AF.Relu)  # relu(0)=0; table preload

    # Params land in 4 packed DMAs (vs 17 individual ones): w0p (A/B input
    # weights), xall (all 4 batches side by side), wbf (W1bd|W2bd), wf
    # (biases + FOLD + head weights).
    w0a = wpool.tile([C, 128], bf16, tag="w0a")
    nc.sync.dma_start(w0a[:], params["w0p"][0][:, 0:128])
    w0b = wpool.tile([C, H], bf16, tag="w0b")
    nc.sync.dma_start(w0b[:], params["w0p"][0][:, 128:160])
    xbs = []
    for b in range(B_PER_CORE):
        xb = xpool.tile([C, L], bf16, tag="xb", name=f"xb{b}")
        nc.sync.dma_start(xb[:], x_in[:, b * L:(b + 1) * L])
        xbs.append(xb)
    wbf = wpool.tile([128, 256], bf16, tag="wbf")
    nc.sync.dma_start(wbf[:], params["wbf"][0][:])
    wf = wpool.tile([128, 101], f32, tag="wf")
    nc.sync.dma_start(wf[:], params["wf"][0][:])

    ld = {
        "W0a4": w0a[:], "W0b": w0b[:],
        "W1bd": wbf[:, 0:128], "W2bd": wbf[:, 128:256],
        "b04": wf[:, 0:1], "b14": wf[:, 1:2], "b24": wf[:, 2:3],
        "FOLD": wf[:, 3:35], "Wp": wf[0:H, 35:67], "bp": wf[0:H, 67:68],
        "Wo": wf[0:H, 68:100], "bo": wf[0:H, 100:101],
    }

    S4 = spool.tile([128, B_PER_CORE], f32, tag="S4")

    # Per-batch setup: AT4pb (A^T + b0, replicated x4, bf16) and BT4
    # (B^T stacked by group, bf16) stay resident for the chunk loop.
    ATs, BTs, ACCs = [], [], []
    for b in range(B_PER_CORE):
        xb = xbs[b][:]
        pA = ps1_pool.tile([128, L], f32, tag="p1")
        nc.tensor.matmul(pA[:], ld["W0a4"], xb, start=True, stop=True)
        AT4pb = atpool.tile([128, L], bf16, tag="AT4pb", name=f"AT4pb{b}")
        nc.scalar.activation(AT4pb[:], pA[:], AF.Identity, bias=ld["b04"])

        pB = ps2_pool.tile([128, TPB], f32, tag="p2")
        for g in range(NG):
            nc.tensor.matmul(
                pB[32 * g:32 * (g + 1), :], ld["W0b"],
                xb[:, TPB * g:TPB * (g + 1)].opt(),
                start=True, stop=True, tile_position=(0, 32 * g))
        BT4 = btpool.tile([128, TPB], f32, tag="BT4", name=f"BT4_{b}")
        nc.vector.tensor_copy(BT4[:], pB[:])

        acc_cols = accpool.tile([128, NCH], f32, tag="acc", name=f"acc{b}")
        ATs.append(AT4pb); BTs.append(BT4); ACCs.append(acc_cols)

    CLS = {"X": "AD", "Y": "DA", "Z": "AA", "W": "DD"}
    if cfg.get("evac_pattern") or cfg.get("act2_pattern"):
        pat2 = cfg.get("act2_pattern") or cfg["evac_pattern"]
        pat3 = cfg.get("act3_pattern") or cfg["evac_pattern"]
    else:
        cp = cfg["chunk_pattern"]
        pat2 = "".join(CLS[c][0] for c in cp)
        pat3 = "".join(CLS[c][1] for c in cp)
    g2 = g3 = 0  # per-stage chunk counters for the engine patterns

    def evac(dst, src, bias, accum=None):
        nonlocal g2, g3
        if accum is not None:
            eng = pat3[g3 % len(pat3)]
            g3 += 1
        else:
            eng = pat2[g2 % len(pat2)]
            g2 += 1
        if eng == "D":
            if accum is None:
                nc.vector.tensor_scalar(dst, src, bias, 0.0, ALU.add, ALU.max)
            else:
                # relu+bias+row-sum in one DVE op: out = (src+bias) max 0,
                # accum_out = sum(out). (tensor_scalar with accum_out would
                # repurpose op1 as the reduce op and drop the relu.)
                nc.vector.scalar_tensor_tensor(
                    dst, src, bias, zcol[:, 0:1].broadcast_to(dst.shape),
                    ALU.add, ALU.max, accum_out=accum)
        else:
            kw = {} if accum is None else {"accum_out": accum}
            nc.scalar.activation(dst, src, AF.Relu, bias=bias, **kw)

    for ch in range(NCH):
        for b in range(B_PER_CORE):
            AT4pb, BT4, acc_cols = ATs[b], BTs[b], ACCs[b]

            h1 = h1pool.tile([128, CW], bf16, tag="h1")
            for k in range(TPC):
                t = ch * TPC + k
                nc.vector.tensor_scalar(
                    h1[:, k * L:(k + 1) * L], AT4pb[:], BT4[:, t:t + 1],
                    0.0, ALU.add, ALU.max)

            p1 = ps1_pool.tile([128, CW], f32, tag="p1")
            for m in range(CW // 512):
                nc.tensor.matmul(
                    p1[:, m * 512:(m + 1) * 512], ld["W1bd"],
                    h1[:, m * 512:(m + 1) * 512], start=True, stop=True)

            h2 = h2pool.tile([128, CW], bf16, tag="h2")
            evac(h2[:], p1[:], ld["b14"])

            p2 = ps2_pool.tile([128, CW], f32, tag="p2")
            for m in range(CW // 512):
                nc.tensor.matmul(
                    p2[:, m * 512:(m + 1) * 512], ld["W2bd"],
                    h2[:, m * 512:(m + 1) * 512], start=True, stop=True)

            if cfg.get("h3_inplace", False):
                # act3's elementwise output is dead data (only the accumulated
                # pair-sums are consumed) -- write it back over p2 in place to
                # skip the SBUF write-port penalty and the h3 SBUF tile.
                evac(p2[:], p2[:], ld["b24"], accum=acc_cols[:, ch:ch + 1])
            else:
                h3 = h3pool.tile([128, CW], bf16, tag="h3")
                evac(h3[:], p2[:], ld["b24"], accum=acc_cols[:, ch:ch + 1])

    for b in range(B_PER_CORE):
        nc.vector.tensor_reduce(
            S4[:, b:b + 1], ACCs[b][:, 0:NCH],
            axis=mybir.AxisListType.X, op=ALU.add)

    pS = ps1_pool.tile([H, B_PER_CORE], f32, tag="p1")
    nc.tensor.matmul(pS[:], ld["FOLD"], S4[:], start=True, stop=True)
    sT = spool.tile([H, B_PER_CORE], f32, tag="sT")
    nc.vector.tensor_copy(sT[:], pS[:])

    pF = ps2_pool.tile([H, B_PER_CORE], f32, tag="p2")
    nc.tensor.matmul(pF[:], ld["Wp"], sT[:], start=True, stop=True)
    fT = spool.tile([H, B_PER_CORE], f32, tag="fT")
    nc.scalar.activation(fT[:], pF[:], AF.Relu, bias=ld["bp"])

    pO = ps1_pool.tile([H, B_PER_CORE], f32, tag="p1")
    nc.tensor.matmul(pO[:], ld["Wo"], fT[:], start=True, stop=True)
    oT = spool.tile([H, B_PER_CORE], f32, tag="oT")
    nc.scalar.activation(oT[:], pO[:], AF.Identity, bias=ld["bo"])

    nc.sync.dma_start(out[:], oT[:])


def _prep_weights(W0, b0, W1, b1, W2, b2, Wp, bp, Wo, bo):
    import ml_dtypes
    bfnp = ml_dtypes.bfloat16
    W0 = np.asarray(W0, np.float32)
    W0a, W0b = W0[:C], W0[C:]
    bd = lambda W: np.kron(np.eye(NG, dtype=np.float32),
                           np.asarray(W, np.float32))
    w0p = np.concatenate(
        [np.tile(W0a, (1, NG)), W0b], axis=1).astype(bfnp)
    wbf = np.concatenate([bd(W1), bd(W2)], axis=1).astype(bfnp)
    wf = np.zeros((128, 101), np.float32)
    wf[:, 0] = np.tile(np.asarray(b0, np.float32), NG)
    wf[:, 1] = np.tile(np.asarray(b1, np.float32), NG)
    wf[:, 2] = np.tile(np.asarray(b2, np.float32), NG)
    wf[:, 3:35] = np.tile(np.eye(H, dtype=np.float32), (NG, 1))
    wf[0:H, 35:67] = np.asarray(Wp, np.float32)
    wf[0:H, 67] = np.asarray(bp, np.float32)
    wf[0:H, 68:100] = np.asarray(Wo, np.float32)
    wf[0:H, 100] = np.asarray(bo, np.float32)
    return {"w0p": np.ascontiguousarray(w0p),
            "wbf": np.ascontiguousarray(wbf), "wf": wf}


def make_in_maps(inputs):
    import ml_dtypes
    bfnp = ml_dtypes.bfloat16
    x_img = inputs["x_img"]
    wd = _prep_weights(**{k: v for k, v in inputs.items() if k != "x_img"})
    x = np.asarray(x_img, np.float32).reshape(B, C, L)
    in_maps = []
    for c in range(N_CORES):
        xc = x[c * B_PER_CORE:(c + 1) * B_PER_CORE]  # [4, C, L]
        xc = np.ascontiguousarray(
            xc.transpose(1, 0, 2).reshape(C, B_PER_CORE * L)).astype(bfnp)
        in_maps.append({"x": xc, **wd})
    return in_maps


def kernel(x_img, W0, b0, W1, b1, W2, b2, Wp, bp, Wo, bo):
    if "nc" not in _CACHE:
        _CACHE["nc"] = _build()
    nc = _CACHE["nc"]

    in_maps = make_in_maps(dict(x_img=x_img, W0=W0, b0=b0, W1=W1, b1=b1,
                                W2=W2, b2=b2, Wp=Wp, bp=bp, Wo=Wo, bo=bo))

    from concourse import bass2jax
    results = bass2jax.run_bass_via_pjrt(nc, in_maps, n_cores=N_CORES)
    full = np.concatenate([r["out"].T for r in results], axis=0)  # [32, 32]
    return full.astype(np.float32)
